# revision 1
# baseline (speedup 1.0000x reference)
"""Trainium2 Bass kernel for dual-branch local+dilated windowed attention.

Problem: B=1, L=4096, D=512, H=8 heads (dh=64), window=±256, dilation=4.
reference returns (out_local, out_dilated), each [1, L, D] fp32.

Sharding: sequence (L) sharded across 8 cores; each core owns 512 query rows
and loads a 1024-row key slice (256-row halo each side, zero-padded at the
sequence edges).  All weights are replicated, pre-transposed, and cast to
bf16 host-side with the rmsnorm gains (and the 1/sqrt(dh) score scale)
folded in.  Per-core key-validity masks fold sequence-edge padding and the
key_padding_mask into the softmax denominator via a masked ones-column
appended to V.

On-chip pipeline per core (single NEFF, SPMD over 8 cores):
  1. rmsnorm(x) -> xhat (bf16), PE-transpose to xhat^T [D_part, t]
  2. Q/K/V projections per branch (PE, bf16, fp32 PSUM accum)
  3. local branch: per 128-key chunk, scores^T = K^T_chunk x Q -> exp (ACT)
     -> edge-triangle {0,1} mask-mults (DVE) -> attn @ [V | colmask] (PE)
     giving per-query numerators + denominator
  4. dilated branch: queries/keys regrouped by residue mod 4 (strided APs),
     where the band collapses to 2 chunks with the same triangle masks
  5. normalize by reciprocal denominator, PE-transpose, Wo projection,
     DMA out (dilated rows are written back through a strided view)
"""

import numpy as np
import ml_dtypes

L, D, H, DH = 4096, 512, 8, 64
WIN, DIL = 256, 4
EPS = 1e-6
NCORES = 8
QL = L // NCORES          # 512 queries per core
KL = QL + 2 * WIN         # 1024 keys per core (halo)
P = 128
NKC = KL // P             # 8 key chunks
NQT = QL // P             # 4 query tiles
BF16 = ml_dtypes.bfloat16
FP8 = ml_dtypes.float8_e4m3fn

_STATE = {}


def _build_nc():
    import concourse.bacc as bacc
    import concourse.tile as tile
    import concourse.mybir as mybir
    from concourse.masks import make_identity
    from concourse.bass import broadcast_tensor_aps

    f32 = mybir.dt.float32
    bf16 = mybir.dt.bfloat16
    Exp = mybir.ActivationFunctionType.Exp
    Ln = mybir.ActivationFunctionType.Ln
    Square = mybir.ActivationFunctionType.Square
    Sqrt = mybir.ActivationFunctionType.Sqrt

    nc = bacc.Bacc()

    fp8 = mybir.dt.float8e4
    DR = mybir.MatmulPerfMode.DoubleRow
    xn = nc.dram_tensor("xn", [KL, D], bf16, kind="ExternalInput")
    wT = {}
    for br in ("l", "d"):
        for w in ("wq", "wk", "wv", "wo"):
            dt_w = fp8 if w in ("wq", "wk") else bf16
            wT[w, br] = nc.dram_tensor(f"{w}T_{br}", [D, D], dt_w,
                                       kind="ExternalInput")
    tri_lo_d = nc.dram_tensor("tri_lo", [P, P], bf16, kind="ExternalInput")
    tri_hi_d = nc.dram_tensor("tri_hi", [P, P], bf16, kind="ExternalInput")
    colmask_d_ = {
        "l": nc.dram_tensor("colmask_l", [P, NKC], f32, kind="ExternalInput"),
        "d": nc.dram_tensor("colmask_d", [P, NKC], f32, kind="ExternalInput"),
    }
    out_dram = {
        "l": nc.dram_tensor("out_l", [QL, D], f32, kind="ExternalOutput"),
        "d": nc.dram_tensor("out_d", [QL, D], f32, kind="ExternalOutput"),
    }

    with tile.TileContext(nc) as tc:
        with (
            tc.tile_pool(name="singles", bufs=1) as singles,
            tc.tile_pool(name="xpool", bufs=3) as xpool,
            tc.tile_pool(name="small", bufs=6) as small,
            tc.tile_pool(name="exppool", bufs=5) as exppool,
            tc.tile_pool(name="outpool", bufs=2) as outpool,
            tc.tile_pool(name="ptr", bufs=2, space="PSUM") as psum_tr,
            tc.tile_pool(name="pproj", bufs=2, space="PSUM") as psum_proj,
            tc.tile_pool(name="pst", bufs=2, space="PSUM") as psum_st,
            tc.tile_pool(name="po", bufs=2, space="PSUM") as psum_o,
        ):
            identity = singles.tile([P, P], bf16)
            make_identity(nc, identity)
            xhatT = singles.tile([P, 4, KL], fp8, name="xhatT")
            xhatTb = singles.tile([P, 4, KL], bf16, name="xhatTb")
            eps_t = singles.tile([P, 1], f32, name="eps")
            nc.vector.memset(eps_t, EPS)

            # x first, as four batched DMAs on the two HWDGE queues (the
            # ~2us per-DMA fixed cost dominates 128KB transfers); weights
            # follow on the same FIFO queues ordered first-needed-first,
            # so x gets the SDMA bandwidth before the 4MB of weights.
            xsb = singles.tile([P, NKC, D], bf16, name="xsb")
            for qt in range(4):
                eng = nc.sync if qt % 2 == 0 else nc.scalar
                eng.dma_start(
                    xsb[:, 2 * qt:2 * (qt + 1), :],
                    xn[qt * 2 * P:(qt + 1) * 2 * P, :].rearrange(
                        "(t p) d -> p t d", p=P))
            w_sb = {}
            w_order = [("wq", "l"), ("wk", "l"), ("wq", "d"), ("wk", "d"),
                       ("wv", "l"), ("wv", "d"), ("wo", "l"), ("wo", "d")]
            for i, (w, br) in enumerate(w_order):
                # all weights on the (compute-idle) sync engine: DMA issues
                # block on sem recycling, which would head-of-line block the
                # ACT engine's rmsnorm work if issued from nc.scalar.
                w_sb[w, br] = singles.tile([P, 4, D],
                                           fp8 if w in ("wq", "wk") else bf16,
                                           name=f"{w}_{br}")
                nc.sync.dma_start(
                    w_sb[w, br],
                    wT[w, br][:, :].rearrange("(ic p) o -> p ic o", p=P),
                )
            tri_lo = singles.tile([P, P], bf16)
            nc.gpsimd.dma_start(tri_lo, tri_lo_d[:, :])
            tri_hi = singles.tile([P, P], bf16)
            nc.gpsimd.dma_start(tri_hi, tri_hi_d[:, :])
            colmask = {}
            for br in ("l", "d"):
                colmask[br] = singles.tile([P, NKC], f32, name=f"cm_{br}")
                nc.gpsimd.dma_start(colmask[br], colmask_d_[br][:, :])

            # ---- rmsnorm + transpose (x^2 sums alternate ACT/DVE) ----
            Mult = mybir.AluOpType.mult
            for tt in range(NKC):
                xt = xsb[:, tt, :]
                sqd = xpool.tile([P, D], bf16, tag="sqd")
                ssum = small.tile([P, 1], f32, tag="ssum")
                if tt % 2 == 0:
                    nc.scalar.activation(sqd, xt, Square, accum_out=ssum)
                else:
                    nc.vector.scalar_tensor_tensor(
                        sqd, xt, 1.0, xt, Mult, Mult, accum_out=ssum)
                rstd = small.tile([P, 1], f32, tag="rstd")
                nc.scalar.activation(rstd, ssum, Sqrt, bias=eps_t, scale=1.0 / D)
                nc.vector.reciprocal(rstd, rstd)
                xh = xpool.tile([P, D], bf16, tag="xh")
                nc.vector.tensor_scalar_mul(xh, xt, rstd)
                tpb = psum_tr.tile([P, NQT * P], bf16, tag="tpb")
                for ic in range(4):
                    nc.tensor.transpose(tpb[:, ic * P:(ic + 1) * P],
                                        xh[:, ic * P:(ic + 1) * P], identity)
                nc.vector.tensor_copy(
                    xhatT[:, :, tt * P:(tt + 1) * P],
                    tpb.rearrange("p (ic q) -> p ic q", ic=4))
                nc.scalar.copy(
                    xhatTb[:, :, tt * P:(tt + 1) * P],
                    tpb.rearrange("p (ic q) -> p ic q", ic=4))

            QT, KT, V, OT = {}, {}, {}, {}
            for br in ("l", "d"):
                QT[br] = singles.tile([P, 4, QL], bf16, name=f"QT_{br}")
                KT[br] = singles.tile([P, 4, KL], bf16, name=f"KT_{br}")
                V[br] = singles.tile([P, NKC, H, DH + 1], bf16, name=f"V_{br}")
                OT[br] = singles.tile([P, 4, QL], bf16, name=f"OT_{br}")

            def key_cols_ap(ic, kc, br):
                # lhsT [128, 128] of xhat^T columns for key chunk kc
                if br == "l":
                    return xhatTb[:, ic, kc * P:(kc + 1) * P]
                rho, s = kc // 2, kc % 2
                return xhatTb[:, ic, :].rearrange(
                    "p (b four) -> p four b", four=DIL)[:, rho, s * P:(s + 1) * P]

            # ---- projections ----
            for br in ("l", "d"):
                for pair in range(4):
                    ps = psum_proj.tile([P, D], f32, tag="pp")
                    for ic in (0, 2):
                        nc.tensor.matmul(
                            ps,
                            w_sb["wq", br][:, ic:ic + 2, pair * P:(pair + 1) * P],
                            xhatT[:, ic:ic + 2, WIN:WIN + QL],
                            start=(ic == 0), stop=(ic == 2), perf_mode=DR)
                    nc.vector.tensor_copy(QT[br][:, pair, :], ps)
                for pair in range(4):
                    for half in range(2):
                        ps = psum_proj.tile([P, D], f32, tag="pp")
                        for ic in (0, 2):
                            nc.tensor.matmul(
                                ps,
                                w_sb["wk", br][:, ic:ic + 2,
                                               pair * P:(pair + 1) * P],
                                xhatT[:, ic:ic + 2, half * D:(half + 1) * D],
                                start=(ic == 0), stop=(ic == 2), perf_mode=DR)
                        nc.vector.tensor_copy(
                            KT[br][:, pair, half * D:(half + 1) * D], ps)
                for kc in range(NKC):
                    ps = psum_proj.tile([P, D], f32, tag="pp")
                    for ic in range(4):
                        nc.tensor.matmul(
                            ps, key_cols_ap(ic, kc, br),
                            w_sb["wv", br][:, ic, :],
                            start=(ic == 0), stop=(ic == 3))
                    nc.scalar.copy(
                        V[br][:, kc, :, 0:DH],
                        ps.rearrange("p (h dv) -> p h dv", h=H))
                    nc.vector.memset(V[br][:, kc, :, DH:DH + 1], 1.0)
                    nc.vector.tensor_scalar_mul(
                        V[br][:, kc], V[br][:, kc], colmask[br][:, kc:kc + 1])

            # ---- attention ----
            # scores^T per key-chunk -> exp -> {0,1} triangle masks -> O^T =
            # [V | colmask]^T @ expS (PE accumulates straight into the
            # transposed-output layout Wo wants; row 64 is the softmax
            # denominator).  Denominator reciprocal is broadcast across the
            # 64 dv partitions with a DRAM-bounce DMA.
            def scores_head(br, h):
                r0, pair = 64 * (h % 2), h // 2
                if br == "l":
                    ex = exppool.tile([P, NKC, QL], bf16, tag="exp")
                    for kc in range(NKC):
                        qlo = max(0, P * (kc - 4))
                        qhi = min(QL, P * kc + P)
                        n = qhi - qlo
                        st = psum_st.tile([P, QL], f32, tag="st")
                        nc.tensor.matmul(
                            st[:, :n],
                            KT[br][r0:r0 + 64, pair, kc * P:(kc + 1) * P],
                            QT[br][r0:r0 + 64, pair, qlo:qhi])
                        nc.scalar.activation(ex[:, kc, qlo:qhi], st[:, :n], Exp,
                                             scale=1.0 / 32768)
                    # edge triangle masks: chunk kc==qtile -> tri_lo at
                    # q-offset 128*kc; chunk kc==qtile+4 -> tri_hi at
                    # 128*(kc-4).  Both strides are 640 elements in the
                    # flat view, so batch into 3 DVE ops instead of 8.
                    exf = ex.rearrange("p a b -> p (a b)")
                    g1 = exf[:, 0:4 * (QL + P)].rearrange(
                        "p (a c) -> p a c", c=QL + P)[:, :, 0:P]
                    g2a = exf[:, 4 * QL:4 * QL + 3 * (QL + P)].rearrange(
                        "p (a c) -> p a c", c=QL + P)[:, :, 0:P]
                    g2b = exf[:, 7 * QL + 3 * P:7 * QL + 4 * P]
                    for g, tri in ((g1, tri_lo), (g2a, tri_hi)):
                        ga, ta = broadcast_tensor_aps(
                            g, tri[:, :].rearrange("p (o b) -> p o b", o=1))
                        nc.vector.tensor_mul(ga, ga, ta)
                    nc.vector.tensor_mul(g2b, g2b, tri_hi)
                else:
                    ex = exppool.tile([P, NKC, P], bf16, tag="expd")
                    # 4 score chunks share one PSUM tile so exp runs as
                    # one wide ACT op instead of 4 narrow ones.
                    for half in range(2):
                        st = psum_st.tile([P, QL], f32, tag="st")
                        for j in range(4):
                            idx = half * 4 + j
                            rho, s = idx // 2, idx % 2
                            ktv = KT[br][r0:r0 + 64, pair, :].rearrange(
                                "p (b four) -> p four b", four=DIL
                            )[:, rho, s * P:(s + 1) * P]
                            qtv = QT[br][r0:r0 + 64, pair, :].rearrange(
                                "p (a four) -> p four a", four=DIL)[:, rho, :]
                            nc.tensor.matmul(st[:, j * P:(j + 1) * P],
                                             ktv, qtv)
                        nc.scalar.activation(
                            ex[:, half * 4:(half + 1) * 4, :], st, Exp,
                            scale=1.0 / 32768)
                    exf = ex.rearrange("p a b -> p (a b)")
                    g1 = exf[:, 0:4 * 2 * P].rearrange(
                        "p (a c) -> p a c", c=2 * P)[:, :, 0:P]
                    g2a = exf[:, P:P + 3 * 2 * P].rearrange(
                        "p (a c) -> p a c", c=2 * P)[:, :, 0:P]
                    g2b = exf[:, 7 * P:8 * P]
                    for g, tri in ((g1, tri_lo), (g2a, tri_hi)):
                        ga, ta = broadcast_tensor_aps(
                            g, tri[:, :].rearrange("p (o b) -> p o b", o=1))
                        nc.vector.tensor_mul(ga, ga, ta)
                    nc.vector.tensor_mul(g2b, g2b, tri_hi)
                return ex

            def av_head(br, h, ex):
                r0, pair = 64 * (h % 2), h // 2
                chunk_sets = ([(t, [t + r for r in range(5)]) for t in range(NQT)]
                              if br == "l" else
                              [(rho, [rho * 2, rho * 2 + 1]) for rho in range(DIL)])
                tpb = psum_tr.tile([P, NQT * P], bf16, tag="tpb")
                for t, kcs in chunk_sets:
                    op = psum_o.tile([P, DH + 1], f32, tag="op")
                    for r, kc in enumerate(kcs):
                        src_cols = (ex[:, kc, t * P:(t + 1) * P] if br == "l"
                                    else ex[:, kc, :])
                        nc.tensor.matmul(
                            op, src_cols, V[br][:, kc, h, :],
                            start=(r == 0), stop=(r == len(kcs) - 1))
                    rcp = small.tile([P, 1], f32, tag="rcp")
                    nc.vector.reciprocal(rcp, op[:, DH:DH + 1])
                    osb = small.tile([P, DH], bf16, tag="osb")
                    nc.vector.tensor_scalar_mul(osb, op[:, 0:DH], rcp)
                    nc.tensor.transpose(tpb[:DH, t * P:(t + 1) * P], osb,
                                        identity)
                nc.vector.tensor_copy(OT[br][r0:r0 + 64, pair, :],
                                      tpb[:DH, :])

            def wo_out(br, t):
                ps = psum_proj.tile([P, D], f32, tag="pp")
                for pair in range(4):
                    nc.tensor.matmul(
                        ps, OT[br][:, pair, t * P:(t + 1) * P],
                        w_sb["wo", br][:, pair, :],
                        start=(pair == 0), stop=(pair == 3))
                ob = outpool.tile([P, D], f32, tag="ob")
                nc.scalar.copy(ob, ps)
                if br == "l":
                    nc.sync.dma_start(out_dram[br][t * P:(t + 1) * P, :], ob)
                else:
                    dst = out_dram[br][:, :].rearrange(
                        "(a four) o -> four a o", four=DIL)[t]
                    nc.sync.dma_start(dst, ob)

            # software-pipeline: emit the next heads' scores before the
            # current head's attnV so the PE never idles on exp/mask latency.
            for br in ("l", "d"):
                exs = [scores_head(br, 0)]
                for h in range(H):
                    if h + 1 < H:
                        exs.append(scores_head(br, h + 1))
                    av_head(br, h, exs[h])
                for t in range(NQT):
                    wo_out(br, t)

    nc.finalize()
    return nc


def _prep_host(x, key_padding_mask, weights):
    """Build the per-core input maps (weights shared across cores)."""
    x = np.asarray(x, dtype=np.float32).reshape(L, D)
    kpm = np.asarray(key_padding_mask).reshape(L).astype(bool)

    shared = {}
    for name, arr in weights.items():
        if name.startswith("wq"):
            shared[name] = np.ascontiguousarray(arr.T * 512.0).astype(FP8)
        elif name.startswith("wk"):
            shared[name] = np.ascontiguousarray(arr.T * 64.0).astype(FP8)
        else:
            shared[name] = np.ascontiguousarray(arr.T).astype(BF16)

    idx = np.arange(P)
    tri_lo = (idx[:, None] >= idx[None, :]).astype(BF16)
    tri_hi = (idx[:, None] <= idx[None, :]).astype(BF16)
    shared["tri_lo"], shared["tri_hi"] = tri_lo, tri_hi

    valid_full = np.zeros(L + 2 * WIN, dtype=np.float32)
    valid_full[WIN:WIN + L] = (~kpm).astype(np.float32)

    in_maps = []
    for c in range(NCORES):
        lo = c * QL - WIN
        xnc = np.zeros((KL, D), dtype=np.float32)
        a, b = max(lo, 0), min(lo + KL, L)
        xnc[a - lo:b - lo] = x[a:b]
        v = valid_full[lo + WIN:lo + WIN + KL]  # validity of keys lo..lo+KL
        cm_l = v.reshape(NKC, P).T.astype(np.float32)
        # dilated chunk idx = rho*2+s holds keys lk = 4*(128*s + p) + rho
        cm_d = np.empty((P, NKC), dtype=np.float32)
        for rho in range(DIL):
            for s in range(2):
                lk = DIL * (P * s + idx) + rho
                cm_d[:, rho * 2 + s] = v[lk]
        m = dict(shared)
        m["xn"] = xnc.astype(BF16)
        m["colmask_l"] = np.ascontiguousarray(cm_l)
        m["colmask_d"] = np.ascontiguousarray(cm_d)
        in_maps.append(m)
    return in_maps


def kernel(x, key_padding_mask, wq_l, wk_l, wv_l, wo_l,
           wq_d, wk_d, wv_d, wo_d, g_q, g_kv, **run_kwargs):
    from concourse.bass_utils import run_bass_kernel_spmd

    g_q = np.asarray(g_q, dtype=np.float32)
    g_kv = np.asarray(g_kv, dtype=np.float32)
    scale = 1.0 / np.sqrt(DH)
    weights = {
        "wqT_l": np.asarray(wq_l, np.float32) * (g_q * scale)[None, :],
        "wkT_l": np.asarray(wk_l, np.float32) * g_kv[None, :],
        "wvT_l": np.asarray(wv_l, np.float32) * g_kv[None, :],
        "woT_l": np.asarray(wo_l, np.float32),
        "wqT_d": np.asarray(wq_d, np.float32) * (g_q * scale)[None, :],
        "wkT_d": np.asarray(wk_d, np.float32) * g_kv[None, :],
        "wvT_d": np.asarray(wv_d, np.float32) * g_kv[None, :],
        "woT_d": np.asarray(wo_d, np.float32),
    }
    in_maps = _prep_host(x, key_padding_mask, weights)

    if "nc" not in _STATE:
        _STATE["nc"] = _build_nc()
    res = run_bass_kernel_spmd(_STATE["nc"], in_maps,
                               core_ids=list(range(NCORES)), **run_kwargs)
    _STATE["last_result"] = res

    out_l = np.concatenate([res.results[c]["out_l"] for c in range(NCORES)],
                           axis=0).reshape(1, L, D)
    out_d = np.concatenate([res.results[c]["out_d"] for c in range(NCORES)],
                           axis=0).reshape(1, L, D)
    return (out_l, out_d)

